# revision 6
# baseline (speedup 1.0000x reference)
"""LSTM layer (exclusive scan over sites) on 8 trn2 NeuronCores.

Problem: inputs (512, 512, 2) f32, Wk (130, 512) f32, b (512,) f32.
  x_shift[:, t] = inputs[:, t-1] (zeros at t=0)
  per step t: ifgo = concat([x_t, h]) @ Wk + b; i,f,g,o = split(ifgo, 4)
  c = sig(f)*c + sig(i)*tanh(g); h = sig(o)*tanh(c); out[:, t] = h

Sharding: data-parallel over batch (64 per core), weights replicated.
Per core the 64-batch is split into 2 independent cohorts of 32 that
interleave on the engines to hide the serial per-step latency chain.

Layout is feature-major: tiles are (128 gate/hidden features, batch).
The x-contribution + bias is pre-accumulated into PSUM in blocks of 8
steps via K=3 matmuls (rows [x0, x1, 1]); the recurrent K=128 matmuls
then accumulate on top (start=False). One sigmoid ACT op per step covers
all 4 gates; tanh(z) is computed as 2*sigmoid(2z)-1 with the factor 2
for the g-gate pre-baked into the weights.
"""

import os
import sys

import numpy as np

if "/opt/trn_rl_repo" not in sys.path:
    sys.path.insert(0, "/opt/trn_rl_repo")

import concourse.bass as bass
import concourse.tile as tile
from concourse import bacc, mybir
from concourse.bass_utils import run_bass_kernel_spmd

F32 = mybir.dt.float32
SIG = mybir.ActivationFunctionType.Sigmoid
MULT = mybir.AluOpType.mult
ADD = mybir.AluOpType.add

NCORE = 8
B = 512
NSTEP = 512
FIN = 2
F = 128
BCORE = B // NCORE          # 64 batch per core
NCOH = 2                    # independent cohorts per core
CB = BCORE // NCOH          # 32 batch per cohort
SBLK = 8                    # steps per x-precompute block
NBLK = NSTEP // SBLK


def build_nc():
    nc = bacc.Bacc(
        "TRN2", target_bir_lowering=False, debug=False, num_devices=NCORE
    )

    wh_d = nc.declare_dram_parameter("wh", [F, 4 * F], F32, isOutput=False)
    wxb_d = nc.declare_dram_parameter("wxb", [3, 4 * F], F32, isOutput=False)
    xslab_d = nc.declare_dram_parameter(
        "xslab", [3 * NCOH, NSTEP * CB], F32, isOutput=False
    )
    out_d = nc.declare_dram_parameter("out", [NSTEP, F, BCORE], F32, isOutput=True)

    with tile.TileContext(nc) as tc:
        with (
            tc.tile_pool(name="const", bufs=1) as constp,
            tc.tile_pool(name="xin", bufs=3) as xinp,
            tc.tile_pool(name="psum", bufs=2, space="PSUM") as psump,
            tc.tile_pool(name="sig", bufs=3) as sigp,
            tc.tile_pool(name="tmp", bufs=3) as tmpp,
            tc.tile_pool(name="state", bufs=1) as statep,
            tc.tile_pool(name="hout", bufs=4) as houtp,
        ):
            wh = constp.tile([F, 4 * F], F32, tag="wh")
            nc.sync.dma_start(out=wh[:], in_=wh_d[:])
            wxb = constp.tile([3, 4 * F], F32, tag="wxb")
            nc.sync.dma_start(out=wxb[:], in_=wxb_d[:])

            c_st = []
            h_prev = []
            for ch in range(NCOH):
                cs = statep.tile([F, CB], F32, tag=f"c{ch}")
                nc.vector.memset(cs[:], 0.0)
                c_st.append(cs)
                h0 = houtp.tile([F, CB], F32, tag=f"h{ch}")
                nc.vector.memset(h0[:], 0.0)
                h_prev.append(h0)

            pt_cur = [None] * NCOH

            for blk in range(NBLK):
                # x-part + bias for the next 8 steps, per cohort
                for ch in range(NCOH):
                    xs = xinp.tile([3, SBLK * CB], F32, tag=f"x{ch}")
                    nc.sync.dma_start(
                        out=xs[:],
                        in_=xslab_d[
                            ch * 3 : (ch + 1) * 3,
                            blk * SBLK * CB : (blk + 1) * SBLK * CB,
                        ],
                    )
                    pt = psump.tile([F, 4, SBLK * CB], F32, tag=f"pt{ch}")
                    pt_cur[ch] = pt
                    for g in range(4):
                        # start=True zeroes the whole 2KB PSUM bank (zero
                        # region), so only the first matmul per bank may set
                        # it; gates 1/3 share banks with gates 0/2.
                        nc.tensor.matmul(
                            out=pt[:, g, :],
                            lhsT=wxb[:, g * F : (g + 1) * F],
                            rhs=xs[:],
                            start=(g % 2 == 0),
                            stop=False,
                            skip_group_check=True,
                        )

                for j in range(SBLK):
                    t = blk * SBLK + j
                    for ch in range(NCOH):
                        pt = pt_cur[ch]
                        js, je = j * CB, (j + 1) * CB
                        for g in range(4):
                            nc.tensor.matmul(
                                out=pt[:, g, js:je],
                                lhsT=wh[:, g * F : (g + 1) * F],
                                rhs=h_prev[ch][:],
                                start=False,
                                stop=(j == SBLK - 1),
                                skip_group_check=True,
                            )
                        # sigmoid over all 4 gates in one op (g pre-scaled x2)
                        s = sigp.tile([F, 4, CB], F32, tag=f"s{ch}")
                        nc.scalar.activation(out=s[:], in_=pt[:, :, js:je], func=SIG)
                        si, sf, sg, so = s[:, 0, :], s[:, 1, :], s[:, 2, :], s[:, 3, :]
                        # tanh(g) = 2*sigmoid(2g) - 1
                        tg = tmpp.tile([F, CB], F32, tag=f"tg{ch}")
                        nc.vector.tensor_scalar(tg[:], sg, 2.0, -1.0, MULT, ADD)
                        t2 = tmpp.tile([F, CB], F32, tag=f"t2{ch}")
                        nc.vector.tensor_tensor(t2[:], sf, c_st[ch][:], MULT)
                        t1 = tmpp.tile([F, CB], F32, tag=f"t1{ch}")
                        nc.vector.tensor_tensor(t1[:], si, tg[:], MULT)
                        nc.vector.tensor_tensor(c_st[ch][:], t2[:], t1[:], ADD)
                        # tanh(c) = 2*sigmoid(2c) - 1
                        sc = tmpp.tile([F, CB], F32, tag=f"sc{ch}")
                        nc.scalar.activation(sc[:], c_st[ch][:], SIG, scale=2.0)
                        tcl = tmpp.tile([F, CB], F32, tag=f"tc{ch}")
                        nc.vector.tensor_scalar(tcl[:], sc[:], 2.0, -1.0, MULT, ADD)
                        h = houtp.tile([F, CB], F32, tag=f"h{ch}")
                        nc.vector.tensor_tensor(h[:], so, tcl[:], MULT)
                        nc.sync.dma_start(
                            out=out_d[t, :, ch * CB : (ch + 1) * CB], in_=h[:]
                        )
                        h_prev[ch] = h
    nc.compile()
    return nc


def prepare_inputs(inputs, Wk, b):
    """Host-side prep: shifted-x slabs per core/cohort, prescaled weights."""
    inputs = np.asarray(inputs, dtype=np.float32)
    Wk = np.asarray(Wk, dtype=np.float32)
    b = np.asarray(b, dtype=np.float32)

    x_shift = np.concatenate(
        [np.zeros((B, 1, FIN), np.float32), inputs[:, :-1, :]], axis=1
    )  # (B, NSTEP, FIN)

    wh = Wk[FIN:, :].copy()  # (128, 512)
    wxb = np.concatenate([Wk[:FIN, :], b[None, :]], axis=0).copy()  # (3, 512)
    # pre-scale the g gate (cols 256:384) by 2 so tanh(g) = 2*sig(2g)-1 works
    wh[:, 2 * F : 3 * F] *= 2.0
    wxb[:, 2 * F : 3 * F] *= 2.0

    in_maps = []
    for core in range(NCORE):
        xc = x_shift[core * BCORE : (core + 1) * BCORE]  # (64, NSTEP, 2)
        slab = np.ones((3 * NCOH, NSTEP * CB), np.float32)
        for ch in range(NCOH):
            xcoh = xc[ch * CB : (ch + 1) * CB]  # (CB, NSTEP, 2)
            # (2, NSTEP, CB) -> rows
            slab[ch * 3 : ch * 3 + 2, :] = xcoh.transpose(2, 1, 0).reshape(
                2, NSTEP * CB
            )
        in_maps.append({"wh": wh, "wxb": wxb, "xslab": slab})
    return in_maps


_trace = bool(int(os.environ.get("KERNEL_TRACE", "0")))
_last_run = {}


def kernel(inputs, Wk, b):
    nc = build_nc()
    in_maps = prepare_inputs(inputs, Wk, b)
    res = run_bass_kernel_spmd(
        nc, in_maps, list(range(NCORE)), trace=_trace
    )
    _last_run["res"] = res
    full = np.empty((B, NSTEP, F), np.float32)
    for core in range(NCORE):
        o = res.results[core]["out"]  # (NSTEP, F, BCORE)
        full[core * BCORE : (core + 1) * BCORE] = o.transpose(2, 0, 1)
    return full


# revision 7
# speedup vs baseline: 1.9105x; 1.9105x over previous
"""LSTM layer (exclusive scan over sites) on 8 trn2 NeuronCores.

Problem: inputs (512, 512, 2) f32, Wk (130, 512) f32, b (512,) f32.
  x_shift[:, t] = inputs[:, t-1] (zeros at t=0)
  per step t: ifgo = concat([x_t, h]) @ Wk + b; i,f,g,o = split(ifgo, 4)
  c = sig(f)*c + sig(i)*tanh(g); h = sig(o)*tanh(c); out[:, t] = h

Sharding: data-parallel over batch (64 per core), weights replicated.
Per core the 64-batch is split into 2 independent cohorts of 32 that
interleave on the engines to hide the serial per-step latency chain.

Layout is feature-major: tiles are (128 gate/hidden features, batch).
The x-contribution + bias is pre-accumulated into PSUM in blocks of 8
steps via K=3 matmuls (rows [x0, x1, 1]); the recurrent K=128 matmuls
then accumulate on top (start=False). Matmul operands are bf16 (PSUM
accumulation stays fp32); the cell state c and all elementwise math
stay fp32. One sigmoid ACT op per step covers all 4 gates; tanh(z) is
computed as 2*sigmoid(2z)-1 with the factor 2 for the g-gate pre-baked
into the weights. Single-input elementwise ops and the h->bf16 cast run
on the otherwise-idle GPSIMD engine.
"""

import os
import sys

import numpy as np

if "/opt/trn_rl_repo" not in sys.path:
    sys.path.insert(0, "/opt/trn_rl_repo")

import ml_dtypes

import concourse.bass as bass
import concourse.tile as tile
from concourse import bacc, mybir
from concourse.bass_utils import run_bass_kernel_spmd

F32 = mybir.dt.float32
BF16 = mybir.dt.bfloat16
SIG = mybir.ActivationFunctionType.Sigmoid
MULT = mybir.AluOpType.mult
ADD = mybir.AluOpType.add

NCORE = 8
B = 512
NSTEP = 512
FIN = 2
F = 128
BCORE = B // NCORE          # 64 batch per core
NCOH = 2                    # independent cohorts per core
CB = BCORE // NCOH          # 32 batch per cohort
SBLK = 8                    # steps per x-precompute block
NBLK = NSTEP // SBLK


def build_nc():
    nc = bacc.Bacc(
        "TRN2", target_bir_lowering=False, debug=False, num_devices=NCORE
    )

    wh_d = nc.declare_dram_parameter("wh", [F, 4 * F], BF16, isOutput=False)
    wxb_d = nc.declare_dram_parameter("wxb", [3, 4 * F], BF16, isOutput=False)
    xslab_d = nc.declare_dram_parameter(
        "xslab", [3 * NCOH, NSTEP * CB], BF16, isOutput=False
    )
    out_d = nc.declare_dram_parameter("out", [NSTEP, F, BCORE], F32, isOutput=True)

    with tile.TileContext(nc) as tc:
        with (
            tc.tile_pool(name="const", bufs=1) as constp,
            tc.tile_pool(name="xin", bufs=3) as xinp,
            tc.tile_pool(name="psum", bufs=2, space="PSUM") as psump,
            tc.tile_pool(name="sig", bufs=3) as sigp,
            tc.tile_pool(name="tmp", bufs=3) as tmpp,
            tc.tile_pool(name="state", bufs=1) as statep,
            tc.tile_pool(name="hout", bufs=4) as houtp,
        ):
            wh = constp.tile([F, 4 * F], BF16, tag="wh", name="wh")
            nc.sync.dma_start(out=wh[:], in_=wh_d[:])
            wxb = constp.tile([3, 4 * F], BF16, tag="wxb", name="wxb")
            nc.sync.dma_start(out=wxb[:], in_=wxb_d[:])

            c_st = []
            h_prev = []
            for ch in range(NCOH):
                cs = statep.tile([F, CB], F32, tag=f"c{ch}", name=f"c{ch}")
                nc.vector.memset(cs[:], 0.0)
                c_st.append(cs)
                h0 = houtp.tile([F, CB], BF16, tag=f"hb{ch}", name=f"hb{ch}")
                nc.vector.memset(h0[:], 0.0)
                h_prev.append(h0)

            pt_cur = [None] * NCOH

            for blk in range(NBLK):
                # x-part + bias for the next 8 steps, per cohort
                for ch in range(NCOH):
                    xs = xinp.tile([3, SBLK * CB], BF16, tag=f"x{ch}", name=f"x{ch}")
                    nc.sync.dma_start(
                        out=xs[:],
                        in_=xslab_d[
                            ch * 3 : (ch + 1) * 3,
                            blk * SBLK * CB : (blk + 1) * SBLK * CB,
                        ],
                    )
                    pt = psump.tile(
                        [F, 4, SBLK * CB], F32, tag=f"pt{ch}", name=f"pt{ch}"
                    )
                    pt_cur[ch] = pt
                    for g in range(4):
                        # start=True zeroes the whole 2KB PSUM bank (zero
                        # region), so only the first matmul per bank may set
                        # it; gates 1/3 share banks with gates 0/2.
                        nc.tensor.matmul(
                            out=pt[:, g, :],
                            lhsT=wxb[:, g * F : (g + 1) * F],
                            rhs=xs[:],
                            start=(g % 2 == 0),
                            stop=False,
                            skip_group_check=True,
                        )

                for j in range(SBLK):
                    t = blk * SBLK + j
                    for ch in range(NCOH):
                        pt = pt_cur[ch]
                        js, je = j * CB, (j + 1) * CB
                        for g in range(4):
                            nc.tensor.matmul(
                                out=pt[:, g, js:je],
                                lhsT=wh[:, g * F : (g + 1) * F],
                                rhs=h_prev[ch][:],
                                start=False,
                                stop=(j == SBLK - 1),
                                skip_group_check=True,
                            )
                        # sigmoid over all 4 gates in one op (g pre-scaled x2)
                        s = sigp.tile([F, 4, CB], F32, tag=f"s{ch}", name=f"s{ch}")
                        nc.scalar.activation(out=s[:], in_=pt[:, :, js:je], func=SIG)
                        si, sf, sg, so = s[:, 0, :], s[:, 1, :], s[:, 2, :], s[:, 3, :]
                        # tanh(g) = 2*sigmoid(2g) - 1  (on GPSIMD)
                        tg = tmpp.tile([F, CB], F32, tag=f"tg{ch}", name=f"tg{ch}")
                        nc.gpsimd.tensor_scalar(tg[:], sg, 2.0, -1.0, MULT, ADD)
                        t2 = tmpp.tile([F, CB], F32, tag=f"t2{ch}", name=f"t2{ch}")
                        nc.vector.tensor_tensor(t2[:], sf, c_st[ch][:], MULT)
                        t1 = tmpp.tile([F, CB], F32, tag=f"t1{ch}", name=f"t1{ch}")
                        nc.vector.tensor_tensor(t1[:], si, tg[:], MULT)
                        nc.vector.tensor_tensor(c_st[ch][:], t2[:], t1[:], ADD)
                        # tanh(c) = 2*sigmoid(2c) - 1
                        sc = tmpp.tile([F, CB], F32, tag=f"sc{ch}", name=f"sc{ch}")
                        nc.scalar.activation(sc[:], c_st[ch][:], SIG, scale=2.0)
                        tcl = tmpp.tile([F, CB], F32, tag=f"tc{ch}", name=f"tc{ch}")
                        nc.gpsimd.tensor_scalar(tcl[:], sc[:], 2.0, -1.0, MULT, ADD)
                        h = houtp.tile([F, CB], F32, tag=f"h{ch}", name=f"h{ch}")
                        nc.vector.tensor_tensor(h[:], so, tcl[:], MULT)
                        hb = houtp.tile([F, CB], BF16, tag=f"hb{ch}", name=f"hb{ch}")
                        nc.gpsimd.tensor_copy(out=hb[:], in_=h[:])
                        nc.sync.dma_start(
                            out=out_d[t, :, ch * CB : (ch + 1) * CB], in_=h[:]
                        )
                        h_prev[ch] = hb
    nc.compile()
    return nc


def prepare_inputs(inputs, Wk, b):
    """Host-side prep: shifted-x slabs per core/cohort, prescaled weights."""
    inputs = np.asarray(inputs, dtype=np.float32)
    Wk = np.asarray(Wk, dtype=np.float32)
    b = np.asarray(b, dtype=np.float32)

    x_shift = np.concatenate(
        [np.zeros((B, 1, FIN), np.float32), inputs[:, :-1, :]], axis=1
    )  # (B, NSTEP, FIN)

    wh = Wk[FIN:, :].copy()  # (128, 512)
    wxb = np.concatenate([Wk[:FIN, :], b[None, :]], axis=0).copy()  # (3, 512)
    # pre-scale the g gate (cols 256:384) by 2 so tanh(g) = 2*sig(2g)-1 works
    wh[:, 2 * F : 3 * F] *= 2.0
    wxb[:, 2 * F : 3 * F] *= 2.0
    wh = wh.astype(ml_dtypes.bfloat16)
    wxb = wxb.astype(ml_dtypes.bfloat16)

    in_maps = []
    for core in range(NCORE):
        xc = x_shift[core * BCORE : (core + 1) * BCORE]  # (64, NSTEP, 2)
        slab = np.ones((3 * NCOH, NSTEP * CB), np.float32)
        for ch in range(NCOH):
            xcoh = xc[ch * CB : (ch + 1) * CB]  # (CB, NSTEP, 2)
            slab[ch * 3 : ch * 3 + 2, :] = xcoh.transpose(2, 1, 0).reshape(
                2, NSTEP * CB
            )
        in_maps.append(
            {"wh": wh, "wxb": wxb, "xslab": slab.astype(ml_dtypes.bfloat16)}
        )
    return in_maps


_trace = bool(int(os.environ.get("KERNEL_TRACE", "0")))
_last_run = {}


def kernel(inputs, Wk, b):
    nc = build_nc()
    in_maps = prepare_inputs(inputs, Wk, b)
    res = run_bass_kernel_spmd(
        nc, in_maps, list(range(NCORE)), trace=_trace
    )
    _last_run["res"] = res
    full = np.empty((B, NSTEP, F), np.float32)
    for core in range(NCORE):
        o = res.results[core]["out"]  # (NSTEP, F, BCORE)
        full[core * BCORE : (core + 1) * BCORE] = o.transpose(2, 0, 1)
    return full


# revision 8
# speedup vs baseline: 2.4427x; 1.2786x over previous
"""LSTM layer (exclusive scan over sites) on 8 trn2 NeuronCores.

Problem: inputs (512, 512, 2) f32, Wk (130, 512) f32, b (512,) f32.
  x_shift[:, t] = inputs[:, t-1] (zeros at t=0)
  per step t: ifgo = concat([x_t, h]) @ Wk + b; i,f,g,o = split(ifgo, 4)
  c = sig(f)*c + sig(i)*tanh(g); h = sig(o)*tanh(c); out[:, t] = h

Sharding: data-parallel over batch (64 per core), weights replicated.
Per core the 64-batch is split into 2 independent cohorts of 32 that
interleave on the engines to hide the serial per-step latency chain.

Layout is feature-major: tiles are (128 gate/hidden features, batch).
Gates are reordered to [i, f, o, g] so one sigmoid op covers i,f,o and
one tanh op covers g (both functions live in the same ACT table set).
The x-contribution + bias is pre-accumulated into PSUM in blocks of 8
steps via K=3 matmuls (rows [x0, x1, 1]); the recurrent K=128 matmuls
then accumulate on top (start=False). Matmul operands are bf16 (PSUM
accumulation stays fp32); the cell state c and gate math stay fp32.
h is written in bf16 into an 8-step staging tile that serves directly
as the next matmul's rhs and is DMA'd out once per block; the host
upconverts the bf16 output to fp32.
"""

import os
import sys

import numpy as np

if "/opt/trn_rl_repo" not in sys.path:
    sys.path.insert(0, "/opt/trn_rl_repo")

import ml_dtypes

import concourse.bass as bass
import concourse.tile as tile
from concourse import bacc, mybir
from concourse.bass_utils import run_bass_kernel_spmd

F32 = mybir.dt.float32
BF16 = mybir.dt.bfloat16
SIG = mybir.ActivationFunctionType.Sigmoid
TANH = mybir.ActivationFunctionType.Tanh
MULT = mybir.AluOpType.mult
ADD = mybir.AluOpType.add

NCORE = 8
B = 512
NSTEP = 512
FIN = 2
F = 128
BCORE = B // NCORE          # 64 batch per core
NCOH = 2                    # independent cohorts per core
CB = BCORE // NCOH          # 32 batch per cohort
SBLK = 8                    # steps per x-precompute block
NBLK = NSTEP // SBLK


def build_nc():
    nc = bacc.Bacc(
        "TRN2", target_bir_lowering=False, debug=False, num_devices=NCORE
    )

    wh_d = nc.declare_dram_parameter("wh", [F, 4 * F], BF16, isOutput=False)
    wxb_d = nc.declare_dram_parameter("wxb", [3, 4 * F], BF16, isOutput=False)
    xslab_d = nc.declare_dram_parameter(
        "xslab", [3 * NCOH, NSTEP * CB], BF16, isOutput=False
    )
    out_d = nc.declare_dram_parameter(
        "out", [NBLK, F, SBLK, BCORE], BF16, isOutput=True
    )

    with tile.TileContext(nc) as tc:
        with (
            tc.tile_pool(name="const", bufs=1) as constp,
            tc.tile_pool(name="xin", bufs=3) as xinp,
            tc.tile_pool(name="psum", bufs=2, space="PSUM") as psump,
            tc.tile_pool(name="sig", bufs=3) as sigp,
            tc.tile_pool(name="tmp", bufs=3) as tmpp,
            tc.tile_pool(name="state", bufs=1) as statep,
            tc.tile_pool(name="hout", bufs=3) as houtp,
        ):
            wh = constp.tile([F, 4 * F], BF16, tag="wh", name="wh")
            nc.sync.dma_start(out=wh[:], in_=wh_d[:])
            wxb = constp.tile([3, 4 * F], BF16, tag="wxb", name="wxb")
            nc.sync.dma_start(out=wxb[:], in_=wxb_d[:])

            c_st = []
            for ch in range(NCOH):
                cs = statep.tile([F, CB], F32, tag=f"c{ch}", name=f"c{ch}")
                nc.vector.memset(cs[:], 0.0)
                c_st.append(cs)

            # h staging: 8 steps of bf16 h per cohort; serves as matmul rhs
            # and as the per-block output DMA source. h(-1) = 0 is slot -1
            # of a zeroed initial tile.
            hst_cur = []
            for ch in range(NCOH):
                hst = houtp.tile(
                    [F, SBLK * CB], BF16, tag=f"hst{ch}", name=f"hst{ch}"
                )
                nc.vector.memset(hst[:], 0.0)
                hst_cur.append(hst)
            h_prev = [hst_cur[ch][:, (SBLK - 1) * CB :] for ch in range(NCOH)]

            pt_cur = [None] * NCOH

            for blk in range(NBLK):
                for ch in range(NCOH):
                    xs = xinp.tile([3, SBLK * CB], BF16, tag=f"x{ch}", name=f"x{ch}")
                    nc.sync.dma_start(
                        out=xs[:],
                        in_=xslab_d[
                            ch * 3 : (ch + 1) * 3,
                            blk * SBLK * CB : (blk + 1) * SBLK * CB,
                        ],
                    )
                    pt = psump.tile(
                        [F, 4, SBLK * CB], F32, tag=f"pt{ch}", name=f"pt{ch}"
                    )
                    pt_cur[ch] = pt
                    for g in range(4):
                        # start=True zeroes the whole 2KB PSUM bank (zero
                        # region), so only the first matmul per bank may set
                        # it; gates 1/3 share banks with gates 0/2.
                        nc.tensor.matmul(
                            out=pt[:, g, :],
                            lhsT=wxb[:, g * F : (g + 1) * F],
                            rhs=xs[:],
                            start=(g % 2 == 0),
                            stop=False,
                            skip_group_check=True,
                        )

                hst_new = []
                for ch in range(NCOH):
                    hst = houtp.tile(
                        [F, SBLK * CB], BF16, tag=f"hst{ch}", name=f"hst{ch}"
                    )
                    hst_new.append(hst)

                for j in range(SBLK):
                    for ch in range(NCOH):
                        pt = pt_cur[ch]
                        js, je = j * CB, (j + 1) * CB
                        for g in range(4):
                            nc.tensor.matmul(
                                out=pt[:, g, js:je],
                                lhsT=wh[:, g * F : (g + 1) * F],
                                rhs=h_prev[ch],
                                start=False,
                                stop=(j == SBLK - 1),
                                skip_group_check=True,
                            )
                        # sigmoid(i, f, o) in one op; tanh(g) in a second
                        s = sigp.tile([F, 3, CB], F32, tag=f"s{ch}", name=f"s{ch}")
                        nc.scalar.activation(
                            out=s[:], in_=pt[:, 0:3, js:je], func=SIG
                        )
                        tgh = tmpp.tile([F, CB], F32, tag=f"tg{ch}", name=f"tg{ch}")
                        nc.scalar.activation(
                            out=tgh[:], in_=pt[:, 3, js:je], func=TANH
                        )
                        si, sf, so = s[:, 0, :], s[:, 1, :], s[:, 2, :]
                        t2 = tmpp.tile([F, CB], F32, tag=f"t2{ch}", name=f"t2{ch}")
                        nc.vector.tensor_tensor(t2[:], sf, c_st[ch][:], MULT)
                        t1 = tmpp.tile([F, CB], F32, tag=f"t1{ch}", name=f"t1{ch}")
                        nc.vector.tensor_tensor(t1[:], si, tgh[:], MULT)
                        nc.vector.tensor_tensor(c_st[ch][:], t2[:], t1[:], ADD)
                        tch = tmpp.tile([F, CB], F32, tag=f"tc{ch}", name=f"tc{ch}")
                        nc.scalar.activation(tch[:], c_st[ch][:], TANH)
                        hsl = hst_new[ch][:, js:je]
                        nc.vector.tensor_tensor(hsl, so, tch[:], MULT)
                        h_prev[ch] = hsl

                for ch in range(NCOH):
                    nc.sync.dma_start(
                        out=out_d[blk, :, :, ch * CB : (ch + 1) * CB],
                        in_=hst_new[ch][:].rearrange(
                            "p (j u) -> p j u", j=SBLK
                        ),
                    )
                    hst_cur[ch] = hst_new[ch]
    nc.compile()
    return nc


def prepare_inputs(inputs, Wk, b):
    """Host-side prep: shifted-x slabs per core/cohort, gate-reordered
    weights (i, f, o, g)."""
    inputs = np.asarray(inputs, dtype=np.float32)
    Wk = np.asarray(Wk, dtype=np.float32)
    b = np.asarray(b, dtype=np.float32)

    x_shift = np.concatenate(
        [np.zeros((B, 1, FIN), np.float32), inputs[:, :-1, :]], axis=1
    )  # (B, NSTEP, FIN)

    # reorder gate columns i,f,g,o -> i,f,o,g
    perm = np.concatenate(
        [np.arange(0, 2 * F), np.arange(3 * F, 4 * F), np.arange(2 * F, 3 * F)]
    )
    wh = Wk[FIN:, perm].astype(ml_dtypes.bfloat16)
    wxb = np.concatenate([Wk[:FIN, :], b[None, :]], axis=0)[:, perm].astype(
        ml_dtypes.bfloat16
    )

    in_maps = []
    for core in range(NCORE):
        xc = x_shift[core * BCORE : (core + 1) * BCORE]  # (64, NSTEP, 2)
        slab = np.ones((3 * NCOH, NSTEP * CB), np.float32)
        for ch in range(NCOH):
            xcoh = xc[ch * CB : (ch + 1) * CB]  # (CB, NSTEP, 2)
            slab[ch * 3 : ch * 3 + 2, :] = xcoh.transpose(2, 1, 0).reshape(
                2, NSTEP * CB
            )
        in_maps.append(
            {"wh": wh, "wxb": wxb, "xslab": slab.astype(ml_dtypes.bfloat16)}
        )
    return in_maps


_trace = bool(int(os.environ.get("KERNEL_TRACE", "0")))
_last_run = {}


def kernel(inputs, Wk, b):
    nc = build_nc()
    in_maps = prepare_inputs(inputs, Wk, b)
    res = run_bass_kernel_spmd(
        nc, in_maps, list(range(NCORE)), trace=_trace
    )
    _last_run["res"] = res
    full = np.empty((B, NSTEP, F), np.float32)
    for core in range(NCORE):
        o = np.asarray(res.results[core]["out"], dtype=np.float32)
        # (NBLK, F, SBLK, BCORE) -> (BCORE, NBLK*SBLK, F)
        full[core * BCORE : (core + 1) * BCORE] = o.transpose(3, 0, 2, 1).reshape(
            BCORE, NSTEP, F
        )
    return full
